# revision 1
# baseline (speedup 1.0000x reference)
"""Bass/Tile TRN2 kernel for per-model-batched causal self-attention.

Problem: x[M,B,S,D], qkv_w[M,D,3D], proj_w[M,D,D] -> out[M,B,S,D]
M=8 models sharded across 8 NeuronCores (embarrassingly parallel).

Per-core design (model m), per batch b:
  xT      = PE-transpose(x_b)  (f32r)               [D,S]
  qkT     = wqkv[:, :1024].T-proj (fp32r matmul)    [1024,S] -> bf16 (q^T,k^T rows)
  V       = x @ wqkv[:, 1024:] (fp32r)              [S,512] -> bf16, +ones col
  st[k,q] = K @ Q^T  (bf16, causal-trimmed,         PSUM f32
            head pairs auto-packed via tile_position)
  p       = exp(st/8)  (ScalarE, bf16 out), diag blocks masked by tri01 mul
  y_aug   = p.T @ V_aug (bf16)  -> y[q,d] + softmax sums in col 64 (PSUM)
  y       = y_aug * (1/sums)  per-partition scalar
  ynT     = PE-transpose(y) (f32r)                  [D,S]
  out     = ynT.T @ wproj (fp32r)

The next batch's load/transpose/projection groups are interleaved into the
attention loop (work queue) so the in-order PE has ready work while the
ScalarE exp chain runs.
"""

import sys

if "/opt/trn_rl_repo" not in sys.path:
    sys.path.insert(0, "/opt/trn_rl_repo")

from contextlib import nullcontext
from functools import partial

import numpy as np

import concourse.bass as bass
import concourse.mybir as mybir
import concourse.tile as tile
from concourse import bacc, bass_utils
from concourse.masks import make_identity, make_upper_triangular

M, B, S, D, H = 8, 4, 512, 512, 8
HD = D // H  # 64
F32 = mybir.dt.float32
F32R = mybir.dt.float32r
BF16 = mybir.dt.bfloat16

N_CORES = 8

_cache = {}


def build_nc(reps=1):
    nc = bacc.Bacc("TRN2", target_bir_lowering=False, debug=False)

    x_d = nc.dram_tensor("x", [B, S, D], F32, kind="ExternalInput")
    wqkv_d = nc.dram_tensor("wqkv", [D, 3 * D], F32, kind="ExternalInput")
    wproj_d = nc.dram_tensor("wproj", [D, D], F32, kind="ExternalInput")
    out_d = nc.dram_tensor("out", [B, S, D], F32, kind="ExternalOutput")

    with tile.TileContext(nc) as tc:
        with (
            tc.tile_pool(name="singles", bufs=1) as singles,
            tc.tile_pool(name="xp", bufs=2) as xpool,
            tc.tile_pool(name="xtp", bufs=3) as xtpool,
            tc.tile_pool(name="qk", bufs=2) as qkpool,
            tc.tile_pool(name="vp", bufs=2) as vpool,
            tc.tile_pool(name="se", bufs=3) as sepool,
            tc.tile_pool(name="yp", bufs=2) as ypool,
            tc.tile_pool(name="ytp", bufs=3) as ytpool,
            tc.tile_pool(name="op", bufs=3) as opool,
            tc.tile_pool(name="rp", bufs=4) as rpool,
            tc.tile_pool(name="ps_mm", bufs=2, space=bass.MemorySpace.PSUM) as ps_mm,
            tc.tile_pool(name="ps_att", bufs=3, space=bass.MemorySpace.PSUM) as ps_att,
        ):
          with tc.For_i(0, reps, 1) if reps > 1 else nullcontext():
            # ---- constants ----
            ident = singles.tile([128, 128], F32)
            make_identity(nc, ident[:])
            ident_r = singles.tile([128, 128], F32R)
            nc.vector.tensor_copy(out=ident_r[:], in_=ident[:])
            tri2 = singles.tile([128, 2, 128], BF16)  # keep-mask (k<=q), x2 heads
            make_upper_triangular(nc, tri2[:, 0, :], val=1.0, diag=True)
            nc.gpsimd.tensor_copy(out=tri2[:, 1, :], in_=tri2[:, 0, :])

            wqkv = singles.tile([128, 4, 3 * D], F32R)
            wproj = singles.tile([128, 4, D], F32R)

            state = {}

            # ---------- stage A (loads + projections), as schedulable groups ----
            def emit_load_x(b):
                x_sb = xpool.tile([128, 4, D], F32R, tag="x", name="xsb")
                for stq in range(4):
                    nc.sync.dma_start(
                        out=x_sb[:, stq, :],
                        in_=x_d.ap().bitcast(F32R)[b][
                            stq * 128 : (stq + 1) * 128, :
                        ],
                    )
                v_sb = vpool.tile([128, 4, H, 66], BF16, tag="v", name="vsb")
                nc.gpsimd.memset(v_sb[:, :, :, 64:65], 1.0)
                state[b] = {"x": x_sb, "xT": [], "qkT": {}, "v": v_sb, "ynT": []}
                if b == 0:
                    # only q/k weight columns gate the first matmuls
                    for dc in range(4):
                        nc.sync.dma_start(
                            out=wqkv[:, dc, 0:1024],
                            in_=wqkv_d.ap().bitcast(F32R)[
                                dc * 128 : (dc + 1) * 128, 0:1024
                            ],
                        )

            def emit_xt_group(b, dc):
                st_ = state[b]
                tp = ps_mm.tile([128, 512], F32, tag="mm", name="tpx")
                for st in range(4):
                    nc.tensor.transpose(
                        tp[:, st * 128 : (st + 1) * 128].bitcast(F32R),
                        st_["x"][:, st, dc * 128 : (dc + 1) * 128],
                        ident_r[:],
                    )
                xt = xtpool.tile([128, 512], F32R, tag=f"xt{dc}", name=f"xt{dc}")
                nc.vector.tensor_copy(out=xt[:], in_=tp[:])
                st_["xT"].append(xt)

            def emit_qkt_group(b, mt):
                st_ = state[b]
                mp = ps_mm.tile([128, 512], F32, tag="mm", name="mp")
                for dc in range(4):
                    nc.tensor.matmul(
                        mp[:],
                        wqkv[:, dc, mt * 128 : (mt + 1) * 128],
                        st_["xT"][dc][:],
                        start=(dc == 0),
                        stop=(dc == 3),
                    )
                qk = qkpool.tile([128, 512], BF16, tag=f"qk{mt}", name=f"qk{mt}")
                nc.vector.tensor_copy(out=qk[:], in_=mp[:])
                st_["qkT"][mt] = qk

            def emit_v_group(b, stt):
                st_ = state[b]
                vp_ps = ps_mm.tile([128, 512], F32, tag="mm", name="vp")
                for dc in range(4):
                    nc.tensor.matmul(
                        vp_ps[:],
                        st_["xT"][dc][:, stt * 128 : (stt + 1) * 128],
                        wqkv[:, dc, 1024:1536],
                        start=(dc == 0),
                        stop=(dc == 3),
                    )
                nc.scalar.copy(
                    out=st_["v"][:, stt, :, 0:64],
                    in_=vp_ps[:].rearrange("p (h e) -> p h e", h=H),
                )

            def emit_late_weights(b):
                # V-columns and the output-projection weights: needed only
                # after the q/k projections, so they load in their shadow
                for dc in range(4):
                    nc.sync.dma_start(
                        out=wqkv[:, dc, 1024:1536],
                        in_=wqkv_d.ap().bitcast(F32R)[
                            dc * 128 : (dc + 1) * 128, 1024:1536
                        ],
                    )
                nc.sync.dma_start(
                    out=wproj[:],
                    in_=wproj_d.ap().bitcast(F32R).rearrange("(c p) o -> p c o", p=128),
                )

            def proj_work(b):
                w = [partial(emit_load_x, b)]
                w += [partial(emit_xt_group, b, dc) for dc in range(4)]
                if b == 0:
                    w.append(partial(emit_late_weights, b))
                w += [partial(emit_qkt_group, b, mt) for mt in (0, 4, 1, 5, 2, 6, 3, 7)]
                w += [partial(emit_v_group, b, stt) for stt in range(4)]
                return w

            # ---------- attention ----------
            def emit_scores(b, hg):
                qkT = state[b]["qkT"]
                h0, h1 = 2 * hg, 2 * hg + 1
                se = sepool.tile([128, 4, 2, 512], BF16, tag="se", name="se")
                for kt in range(4):
                    off = 128 * kt
                    stp = ps_att.tile([128, 1024], F32, tag="att", name="stp")
                    for hi, h in enumerate((h0, h1)):
                        mtq, poq = h // 2, 64 * (h % 2)
                        mtk, pok = 4 + h // 2, 64 * (h % 2)
                        nc.tensor.matmul(
                            stp[:, hi * 512 + off : hi * 512 + 512],
                            qkT[mtk][pok : pok + 64, kt * 128 : (kt + 1) * 128],
                            qkT[mtq][poq : poq + 64, off:512],
                            start=True,
                            stop=True,
                        )
                    nc.scalar.activation(
                        out=se[:, kt, :, off:],
                        in_=stp[:].rearrange("p (hh q) -> p hh q", hh=2)[:, :, off:],
                        func=mybir.ActivationFunctionType.Exp,
                        scale=1.0 / np.sqrt(HD),
                    )
                    # mask the diagonal block (strict lower triangle -> 0)
                    nc.vector.tensor_mul(
                        out=se[:, kt, :, off : off + 128],
                        in0=se[:, kt, :, off : off + 128],
                        in1=tri2[:],
                    )
                return se

            def emit_y(b, hg, se, y_sb):
                st_ = state[b]
                h0, h1 = 2 * hg, 2 * hg + 1
                yp = ps_att.tile([128, 1024], F32, tag="att", name="yp")
                for hi, h in enumerate((h0, h1)):
                    for qt in range(4):
                        base = hi * 512 + qt * 65
                        for kt in range(qt + 1):
                            nc.tensor.matmul(
                                yp[:, base : base + 65],
                                se[:, kt, hi, qt * 128 : (qt + 1) * 128],
                                st_["v"][:, kt, h, 0:65],
                                start=(kt == 0),
                                stop=(kt == qt),
                            )
                rs = rpool.tile([128, 2, 4], F32, tag="rs", name="rs")
                nc.vector.reciprocal_approx_fast(
                    out=rs[:],
                    in_=yp[:].rearrange("p (hh q) -> p hh q", hh=2)[:, :, 64:260:65],
                )
                for hi, h in enumerate((h0, h1)):
                    for qt in range(4):
                        base = hi * 512 + qt * 65
                        nc.vector.tensor_scalar_mul(
                            y_sb[qt][:, 64 * h : 64 * h + 64],
                            yp[:, base : base + 64],
                            rs[:, hi, qt : qt + 1],
                        )
                # yT transpose for the d-slice this head-pair completed
                dc = hg
                tp = ps_mm.tile([128, 512], F32, tag="mm", name="tpy")
                for qt in range(4):
                    nc.tensor.transpose(
                        tp[:, qt * 128 : (qt + 1) * 128].bitcast(F32R),
                        y_sb[qt][:, dc * 128 : (dc + 1) * 128],
                        ident_r[:],
                    )
                yt = ytpool.tile([128, 512], F32R, tag=f"yt{dc}", name=f"yt{dc}")
                nc.scalar.copy(out=yt[:], in_=tp[:])
                st_["ynT"].append(yt)

            def emit_proj_group(b, qt):
                ynT = state[b]["ynT"]
                op_ps = ps_mm.tile([128, 512], F32, tag="mm", name="op")
                for dc in range(4):
                    nc.tensor.matmul(
                        op_ps[:],
                        ynT[dc][:, qt * 128 : (qt + 1) * 128],
                        wproj[:, dc, :],
                        start=(dc == 0),
                        stop=(dc == 3),
                    )
                ob = opool.tile([128, 512], F32, tag="ob", name="ob")
                nc.vector.tensor_copy(out=ob[:], in_=op_ps[:])
                nc.sync.dma_start(
                    out=out_d.ap()[b, qt * 128 : (qt + 1) * 128, :], in_=ob[:]
                )

            # ---------- main schedule ----------
            w0 = proj_work(0)
            for f in w0[:8]:
                f()  # load x0, xT, late-weight DMAs, qkT for head-pair 0
            se_prev = emit_scores(0, 0)
            for f in w0[8:]:
                f()
            pending_proj = []
            for b in range(B):
                queue = (proj_work(b + 1) if b + 1 < B else []) + pending_proj
                y_sb = [
                    ypool.tile([128, 512], F32R, tag=f"y{qt}", name=f"ysb{qt}")
                    for qt in range(4)
                ]
                for hg in range(4):
                    se_next = emit_scores(b, hg + 1) if hg + 1 < 4 else None
                    # fill PE while ScalarE runs the exp chain for this hg
                    for _ in range(6):
                        if queue:
                            queue.pop(0)()
                    emit_y(b, hg, se_prev, y_sb)
                    se_prev = se_next
                while queue:
                    queue.pop(0)()
                # first scores of the next batch fill the normalize/transpose tail
                se_prev = emit_scores(b + 1, 0) if b + 1 < B else None
                # this batch's projection is deferred into the next attention
                pending_proj = [partial(emit_proj_group, b, qt) for qt in range(4)]
            for f in pending_proj:
                f()

    nc.compile()
    return nc


def kernel(x, qkv_weight, proj_weight):
    if "nc" not in _cache:
        _cache["nc"] = build_nc()
    nc = _cache["nc"]
    in_maps = [
        {
            "x": np.ascontiguousarray(x[m], dtype=np.float32),
            "wqkv": np.ascontiguousarray(qkv_weight[m], dtype=np.float32),
            "wproj": np.ascontiguousarray(proj_weight[m], dtype=np.float32),
        }
        for m in range(M)
    ]
    res = bass_utils.run_bass_kernel_spmd(nc, in_maps, core_ids=list(range(N_CORES)))
    return np.stack([res.results[m]["out"] for m in range(M)]).astype(np.float32)



# revision 27
# speedup vs baseline: 1.2023x; 1.2023x over previous
"""Bass/Tile TRN2 kernel for per-model-batched causal self-attention.

Problem: x[M,B,S,D], qkv_w[M,D,3D], proj_w[M,D,D] -> out[M,B,S,D]
M=8 models sharded across 8 NeuronCores (embarrassingly parallel).

Per-core dataflow (model m, per batch b):
  xb      = cast-DMA x -> bf16                        [S,D] bf16
  xT      = PE-transpose(xb) (bf16, 1 cyc/row)        [D,S] bf16
  xdr     = fp8(xT) interleaved [128, 2, 512] pairs for DoubleRow
  qkT     = fp8 DoubleRow matmul w_qk_dr.T @ xdr      [1024,S] -> bf16
            (2 accumulation steps of 256-contraction, 0.5 cyc/row)
  V       = xT.T @ w_v (bf16)                         [S,512] bf16, +ones col
  st[k,q] = K @ Q^T (bf16, causal-trimmed, head pairs row-packed)
  p       = exp(st/8) (ScalarE), diag blocks masked by tri01 mul (DVE)
  y_aug   = p.T @ V_aug (bf16) -> y[q,d] + softmax sums in col 64
  y       = y_aug * (1/sums)  one broadcast tensor_tensor per head pair
  ynT     = PE-transpose(y) (bf16)                    [D,S]
  out     = ynT.T @ w_p (bf16)

Scheduling: the next batch's load/transpose/projection groups and the
previous batch's output projections are interleaved into the attention
loop as a work queue, paced (pop plan per head-pair) so each group lands
after its inputs exist and the in-order PE never blocks on a producer.
Each head-pair's yT transposes are deferred one slot so the PE never
waits on the recip/normalize chain. q/k weights load as f32 on the HWDGE
queue (parallel with the SWDGE x casts), split so the mt0/mt4 columns
convert to fp8 first and unblock the first score tile.
"""

import sys

if "/opt/trn_rl_repo" not in sys.path:
    sys.path.insert(0, "/opt/trn_rl_repo")

from contextlib import nullcontext
from functools import partial

import numpy as np

import concourse.bass as bass
import concourse.mybir as mybir
import concourse.tile as tile
from concourse import bacc, bass_utils
from concourse.masks import make_identity, make_upper_triangular

M, B, S, D, H = 8, 4, 512, 512, 8
HD = D // H  # 64
F32 = mybir.dt.float32
BF16 = mybir.dt.bfloat16
FP8 = mybir.dt.float8e4
DR = mybir.MatmulPerfMode.DoubleRow

N_CORES = 8

_cache = {}


def build_nc(reps=1):
    nc = bacc.Bacc("TRN2", target_bir_lowering=False, debug=False)

    x_d = nc.dram_tensor("x", [B, S, D], F32, kind="ExternalInput")
    wqkv_d = nc.dram_tensor("wqkv", [D, 3 * D], F32, kind="ExternalInput")
    wproj_d = nc.dram_tensor("wproj", [D, D], F32, kind="ExternalInput")
    out_d = nc.dram_tensor("out", [B, S, D], F32, kind="ExternalOutput")

    with tile.TileContext(nc) as tc:
        with (
            tc.tile_pool(name="singles", bufs=1) as singles,
            tc.tile_pool(name="xbp", bufs=2) as xbpool,
            tc.tile_pool(name="xtp", bufs=2) as xtpool,
            tc.tile_pool(name="xdp", bufs=2) as xdpool,
            tc.tile_pool(name="qk", bufs=2) as qkpool,
            tc.tile_pool(name="vp", bufs=2) as vpool,
            tc.tile_pool(name="se", bufs=3) as sepool,
            tc.tile_pool(name="yp", bufs=2) as ypool,
            tc.tile_pool(name="ytp", bufs=3) as ytpool,
            tc.tile_pool(name="op", bufs=3) as opool,
            tc.tile_pool(name="rp", bufs=4) as rpool,
            tc.tile_pool(name="ps_mm", bufs=2, space=bass.MemorySpace.PSUM) as ps_mm,
            tc.tile_pool(name="ps_att", bufs=3, space=bass.MemorySpace.PSUM) as ps_att,
        ):
          with tc.For_i(0, reps, 1) if reps > 1 else nullcontext():
            # ---- constants ----
            identf = singles.tile([128, 128], F32)
            make_identity(nc, identf[:])
            ident = singles.tile([128, 128], BF16)
            nc.vector.tensor_copy(out=ident[:], in_=identf[:])
            tri2 = singles.tile([128, 2, 128], BF16)  # keep-mask (k<=q), x2 heads
            make_upper_triangular(nc, tri2[:, 0, :], val=1.0, diag=True)
            nc.gpsimd.tensor_copy(out=tri2[:, 1, :], in_=tri2[:, 0, :])

            wqk_f32 = singles.tile([128, 4, 1024], F32)
            w_qk_dr = singles.tile([128, 2, 2, 1024], FP8)
            w_v = singles.tile([128, 4, 512], BF16)
            w_p = singles.tile([128, 4, 512], BF16)

            state = {}

            # qk-weight column halves: A = mt chunks 0 and 4 (unblocks the
            # first score tile), B = the rest. f32 via HWDGE, parallel with
            # the SWDGE x casts; fp8 conversion on DVE.
            QK_A = ((0, 128), (512, 640))
            QK_B = ((128, 512), (640, 1024))

            def emit_qk_weights(ranges):
                for dc in range(4):
                    for lo, hi in ranges:
                        nc.scalar.dma_start(
                            out=wqk_f32[:, dc, lo:hi],
                            in_=wqkv_d.ap()[dc * 128 : (dc + 1) * 128, lo:hi],
                        )

            def emit_qk_weight_conv(ranges, eng):
                for dpair in range(2):
                    for t in range(2):
                        for lo, hi in ranges:
                            eng(
                                out=w_qk_dr[:, dpair, t, lo:hi],
                                in_=wqk_f32[:, 2 * dpair + t, lo:hi],
                            )

            def emit_late_weights():
                # V-columns and output-projection weights: needed only after
                # the q/k projections, so they load in their shadow
                for dc in range(4):
                    nc.gpsimd.dma_start(
                        out=w_v[:, dc, :],
                        in_=wqkv_d.ap()[dc * 128 : (dc + 1) * 128, 1024:1536],
                    )
                nc.gpsimd.dma_start(
                    out=w_p[:],
                    in_=wproj_d.ap().rearrange("(c p) o -> p c o", p=128),
                )

            def emit_load_x(b):
                # b0 loads per dc-column-chunk so each transpose group can
                # start as soon as its quarter arrives; later batches have a
                # full slot of slack, so one DMA saves Q7 descriptor time
                xb = xbpool.tile([128, 4, D], BF16, tag="xb", name="xb")
                xsrc = x_d.ap()[b].rearrange("(st p) d -> p st d", p=128)
                if b == 0:
                    for dc in range(4):
                        nc.gpsimd.dma_start(
                            out=xb[:, :, dc * 128 : (dc + 1) * 128],
                            in_=xsrc[:, :, dc * 128 : (dc + 1) * 128],
                        )
                else:
                    nc.gpsimd.dma_start(out=xb[:], in_=xsrc)
                v_sb = vpool.tile([128, 4, H, 66], BF16, tag="v", name="vsb")
                nc.gpsimd.memset(v_sb[:, :, :, 64:65], 1.0)
                state[b] = {"xb": xb, "xT": [], "xdr": {}, "qkT": {}, "v": v_sb,
                            "ynT": []}

            def emit_xt_group(b, dc):
                st_ = state[b]
                tp = ps_mm.tile([128, 512], F32, tag="mm", name="tpx")
                tb = tp[:].bitcast(BF16)  # [128, 1024]
                for st in range(4):
                    nc.tensor.transpose(
                        tb[:, st * 128 : (st + 1) * 128],
                        st_["xb"][:, st, dc * 128 : (dc + 1) * 128],
                        ident[:],
                    )
                xt = xtpool.tile([128, 512], BF16, tag=f"xt{dc}", name=f"xt{dc}")
                nc.vector.tensor_copy(out=xt[:], in_=tb[:, 0:512])
                st_["xT"].append(xt)
                dpair = dc // 2
                if dc % 2 == 0:
                    st_["xdr"][dpair] = xdpool.tile(
                        [128, 2, 512], FP8, tag=f"xdr{dpair}", name=f"xdr{dpair}"
                    )
                nc.gpsimd.tensor_copy(
                    out=st_["xdr"][dpair][:, dc % 2, :], in_=xt[:]
                )

            def emit_qkt_group(b, mt):
                st_ = state[b]
                mp = ps_mm.tile([128, 512], F32, tag="mm", name="mp")
                for dpair in range(2):
                    nc.tensor.matmul(
                        mp[:],
                        w_qk_dr[:, dpair, :, mt * 128 : (mt + 1) * 128],
                        st_["xdr"][dpair][:],
                        start=(dpair == 0),
                        stop=(dpair == 1),
                        perf_mode=DR,
                    )
                qk = qkpool.tile([128, 512], BF16, tag=f"qk{mt}", name=f"qk{mt}")
                nc.vector.tensor_copy(out=qk[:], in_=mp[:])
                st_["qkT"][mt] = qk

            def emit_v_group(b, stt):
                st_ = state[b]
                vp_ps = ps_mm.tile([128, 512], F32, tag="mm", name="vp")
                for dc in range(4):
                    nc.tensor.matmul(
                        vp_ps[:],
                        st_["xT"][dc][:, stt * 128 : (stt + 1) * 128],
                        w_v[:, dc, :],
                        start=(dc == 0),
                        stop=(dc == 3),
                    )
                nc.scalar.copy(
                    out=st_["v"][:, stt, :, 0:64],
                    in_=vp_ps[:].rearrange("p (h e) -> p h e", h=H),
                )

            def proj_work(b):
                w = [partial(emit_load_x, b)]
                w += [partial(emit_xt_group, b, dc) for dc in range(4)]
                w += [partial(emit_qkt_group, b, mt) for mt in (0, 4, 1, 5, 2, 6, 3, 7)]
                w += [partial(emit_v_group, b, stt) for stt in range(4)]
                return w

            # ---------- attention ----------
            def emit_scores(b, hg):
                qkT = state[b]["qkT"]
                h0, h1 = 2 * hg, 2 * hg + 1
                se = sepool.tile([128, 4, 2, 512], BF16, tag="se", name="se")
                for kt in range(4):
                    off = 128 * kt
                    stp = ps_att.tile([128, 1024], F32, tag="att", name="stp")
                    for hi, h in enumerate((h0, h1)):
                        mtq, poq = h // 2, 64 * (h % 2)
                        mtk, pok = 4 + h // 2, 64 * (h % 2)
                        nc.tensor.matmul(
                            stp[:, hi * 512 + off : hi * 512 + 512],
                            qkT[mtk][pok : pok + 64, kt * 128 : (kt + 1) * 128],
                            qkT[mtq][poq : poq + 64, off:512],
                            start=True,
                            stop=True,
                        )
                    nc.scalar.activation(
                        out=se[:, kt, :, off:],
                        in_=stp[:].rearrange("p (hh q) -> p hh q", hh=2)[:, :, off:],
                        func=mybir.ActivationFunctionType.Exp,
                        scale=1.0 / np.sqrt(HD),
                    )
                    # mask the diagonal block (strict lower triangle -> 0)
                    nc.vector.tensor_mul(
                        out=se[:, kt, :, off : off + 128],
                        in0=se[:, kt, :, off : off + 128],
                        in1=tri2[:],
                    )
                return se

            def emit_y(b, hg, se, y_sb):
                st_ = state[b]
                h0, h1 = 2 * hg, 2 * hg + 1
                yp = ps_att.tile([128, 1024], F32, tag="att", name="yp")
                for hi, h in enumerate((h0, h1)):
                    for qt in range(4):
                        base = hi * 512 + qt * 65
                        for kt in range(qt + 1):
                            nc.tensor.matmul(
                                yp[:, base : base + 65],
                                se[:, kt, hi, qt * 128 : (qt + 1) * 128],
                                st_["v"][:, kt, h, 0:65],
                                start=(kt == 0),
                                stop=(kt == qt),
                            )
                rs = rpool.tile([128, 2, 4], F32, tag="rs", name="rs")
                nc.vector.reciprocal_approx_fast(
                    out=rs[:],
                    in_=yp[:].rearrange("p (hh q) -> p hh q", hh=2)[:, :, 64:260:65],
                )
                # normalize the whole head-pair with one broadcast tensor_tensor:
                # out[p, hh, qt, c] = yp[p, hh*512 + qt*65 + c] * rs[p, hh, qt]
                yp_v = bass.AP(
                    tensor=yp[:].tensor,
                    offset=yp[:].offset,
                    ap=[yp[:].ap[0], [512, 2], [65, 4], [1, 64]],
                )
                rs_v = bass.AP(
                    tensor=rs[:].tensor,
                    offset=rs[:].offset,
                    ap=[rs[:].ap[0], rs[:].ap[1], rs[:].ap[2], [0, 64]],
                )
                y_v = y_sb[:, :, hg * 128 : (hg + 1) * 128].rearrange(
                    "p q (hh c) -> p hh q c", hh=2
                )
                nc.vector.tensor_mul(out=y_v, in0=yp_v, in1=rs_v)

            def emit_y_tail(b, hg, y_sb):
                st_ = state[b]
                dc = hg
                tp = ps_mm.tile([128, 512], F32, tag="mm", name="tpy")
                tb = tp[:].bitcast(BF16)
                for qt in range(4):
                    nc.tensor.transpose(
                        tb[:, qt * 128 : (qt + 1) * 128],
                        y_sb[:, qt, dc * 128 : (dc + 1) * 128],
                        ident[:],
                    )
                yt = ytpool.tile([128, 512], BF16, tag=f"yt{dc}", name=f"yt{dc}")
                nc.scalar.copy(out=yt[:], in_=tb[:, 0:512])
                st_["ynT"].append(yt)

            def emit_proj_group(b, qt):
                ynT = state[b]["ynT"]
                op_ps = ps_mm.tile([128, 512], F32, tag="mm", name="op")
                for dc in range(4):
                    nc.tensor.matmul(
                        op_ps[:],
                        ynT[dc][:, qt * 128 : (qt + 1) * 128],
                        w_p[:, dc, :],
                        start=(dc == 0),
                        stop=(dc == 3),
                    )
                ob = opool.tile([128, 512], F32, tag="ob", name="ob")
                nc.vector.tensor_copy(out=ob[:], in_=op_ps[:])
                nc.sync.dma_start(
                    out=out_d.ap()[b, qt * 128 : (qt + 1) * 128, :], in_=ob[:]
                )

            # ---------- main schedule ----------
            w0 = proj_work(0)
            w0[0]()  # load x0 (SWDGE; weight DMAs go on the HWDGE queue)
            emit_qk_weights(QK_A)
            emit_qk_weight_conv(QK_A, nc.vector.tensor_copy)
            for f in w0[1:5]:
                f()  # xT groups
            w0[5]()  # qkT mt0
            w0[6]()  # qkT mt4
            emit_qk_weights(QK_B)
            emit_qk_weight_conv(QK_B, nc.vector.tensor_copy)
            emit_late_weights()
            se_prev = emit_scores(0, 0)
            for f in w0[7:]:
                f()
            pending_proj = []
            for b in range(B):
                if b + 1 < B:
                    w = proj_work(b + 1)
                    queue = w[0:1] + pending_proj + w[1:]
                    plan = (1, 4, 6, 6) if b == 0 else (3, 4, 6, 6)
                else:
                    queue = list(pending_proj)
                    plan = (1, 1, 1, 1)
                y_sb = ypool.tile([128, 4, 512], BF16, tag="y", name="ysb")
                for hg in range(4):
                    se_next = emit_scores(b, hg + 1) if hg + 1 < 4 else None
                    for _ in range(plan[hg]):
                        if queue:
                            queue.pop(0)()
                    if hg > 0:
                        emit_y_tail(b, hg - 1, y_sb)
                    emit_y(b, hg, se_prev, y_sb)
                    se_prev = se_next
                while queue:
                    queue.pop(0)()
                emit_y_tail(b, 3, y_sb)
                # first scores of the next batch fill the normalize/transpose tail
                se_prev = emit_scores(b + 1, 0) if b + 1 < B else None
                # this batch's projection is deferred into the next attention
                pending_proj = [partial(emit_proj_group, b, qt) for qt in range(4)]
            for f in pending_proj:
                f()

    nc.compile()
    return nc


def kernel(x, qkv_weight, proj_weight):
    if "nc" not in _cache:
        _cache["nc"] = build_nc()
    nc = _cache["nc"]
    in_maps = [
        {
            "x": np.ascontiguousarray(x[m], dtype=np.float32),
            "wqkv": np.ascontiguousarray(qkv_weight[m], dtype=np.float32),
            "wproj": np.ascontiguousarray(proj_weight[m], dtype=np.float32),
        }
        for m in range(M)
    ]
    res = bass_utils.run_bass_kernel_spmd(nc, in_maps, core_ids=list(range(N_CORES)))
    return np.stack([res.results[m]["out"] for m in range(M)]).astype(np.float32)


# revision 29
# speedup vs baseline: 1.2857x; 1.0693x over previous
"""Bass/Tile TRN2 kernel for per-model-batched causal self-attention.

Problem: x[M,B,S,D], qkv_w[M,D,3D], proj_w[M,D,D] -> out[M,B,S,D]
M=8 models sharded across 8 NeuronCores (embarrassingly parallel).

Per-core dataflow (model m, per batch b):
  xb      = cast-DMA x -> bf16                        [S,D] bf16
  xT      = PE-transpose(xb) (bf16, 1 cyc/row)        [D,S] bf16
  xdr     = fp8(xT) interleaved [128, 2, 512] pairs for DoubleRow
  qkT     = fp8 DoubleRow matmul w_qk_dr.T @ xdr      [1024,S] -> bf16
            (2 accumulation steps of 256-contraction, 0.5 cyc/row)
  V       = xT.T @ w_v (bf16)                         [S,512] bf16, +ones col
  st[k,q] = K @ Q^T (bf16, causal-trimmed, head pairs row-packed)
  p       = exp(st/8) (ScalarE), diag blocks masked by tri01 mul (DVE)
  y_aug   = p.T @ V_aug (bf16) -> y[q,d] + softmax sums in col 64
  y       = y_aug * (1/sums)  one broadcast tensor_tensor per head pair
  ynT     = PE-transpose(y) (bf16)                    [D,S]
  out     = ynT.T @ w_p (bf16)

Scheduling: the next batch's load/transpose/projection groups and the
previous batch's output projections are interleaved into the attention
loop as a work queue, paced (pop plan per head-pair) so each group lands
after its inputs exist and the in-order PE never blocks on a producer.
Each head-pair's yT transposes are deferred one slot so the PE never
waits on the recip/normalize chain. q/k weights load as f32 on the HWDGE
queue (parallel with the SWDGE x casts), split so the mt0/mt4 columns
convert to fp8 first and unblock the first score tile. The fp8 xdr pairs
are copied straight out of the transpose PSUM on DVE (before the bf16 xT
copy) so the DoubleRow q/k matmuls never wait on a two-hop DVE->Pool
chain.
"""

import sys

if "/opt/trn_rl_repo" not in sys.path:
    sys.path.insert(0, "/opt/trn_rl_repo")

from contextlib import nullcontext
from functools import partial

import numpy as np

import concourse.bass as bass
import concourse.mybir as mybir
import concourse.tile as tile
from concourse import bacc, bass_utils
from concourse.masks import make_identity, make_upper_triangular

M, B, S, D, H = 8, 4, 512, 512, 8
HD = D // H  # 64
F32 = mybir.dt.float32
BF16 = mybir.dt.bfloat16
FP8 = mybir.dt.float8e4
DR = mybir.MatmulPerfMode.DoubleRow

N_CORES = 8

_cache = {}


def build_nc(reps=1):
    nc = bacc.Bacc("TRN2", target_bir_lowering=False, debug=False)

    x_d = nc.dram_tensor("x", [B, S, D], F32, kind="ExternalInput")
    wqkv_d = nc.dram_tensor("wqkv", [D, 3 * D], F32, kind="ExternalInput")
    wproj_d = nc.dram_tensor("wproj", [D, D], F32, kind="ExternalInput")
    out_d = nc.dram_tensor("out", [B, S, D], F32, kind="ExternalOutput")

    with tile.TileContext(nc) as tc:
        with (
            tc.tile_pool(name="singles", bufs=1) as singles,
            tc.tile_pool(name="xbp", bufs=2) as xbpool,
            tc.tile_pool(name="xtp", bufs=2) as xtpool,
            tc.tile_pool(name="xdp", bufs=2) as xdpool,
            tc.tile_pool(name="qk", bufs=2) as qkpool,
            tc.tile_pool(name="vp", bufs=2) as vpool,
            tc.tile_pool(name="se", bufs=3) as sepool,
            tc.tile_pool(name="yp", bufs=2) as ypool,
            tc.tile_pool(name="ytp", bufs=3) as ytpool,
            tc.tile_pool(name="op", bufs=3) as opool,
            tc.tile_pool(name="rp", bufs=4) as rpool,
            tc.tile_pool(name="ps_mm", bufs=2, space=bass.MemorySpace.PSUM) as ps_mm,
            tc.tile_pool(name="ps_att", bufs=3, space=bass.MemorySpace.PSUM) as ps_att,
        ):
          with tc.For_i(0, reps, 1) if reps > 1 else nullcontext():
            # ---- constants ----
            identf = singles.tile([128, 128], F32)
            make_identity(nc, identf[:])
            ident = singles.tile([128, 128], BF16)
            nc.vector.tensor_copy(out=ident[:], in_=identf[:])
            tri2 = singles.tile([128, 2, 128], BF16)  # keep-mask (k<=q), x2 heads
            make_upper_triangular(nc, tri2[:, 0, :], val=1.0, diag=True)
            nc.gpsimd.tensor_copy(out=tri2[:, 1, :], in_=tri2[:, 0, :])

            wqk_f32 = singles.tile([128, 4, 1024], F32)
            w_qk_dr = singles.tile([128, 2, 2, 1024], FP8)
            w_v = singles.tile([128, 4, 512], BF16)
            w_p = singles.tile([128, 4, 512], BF16)

            state = {}

            # qk-weight column halves: A = mt chunks 0 and 4 (unblocks the
            # first score tile), B = the rest. f32 via HWDGE, parallel with
            # the SWDGE x casts; fp8 conversion on DVE.
            QK_A = ((0, 128), (512, 640))
            QK_B = ((128, 512), (640, 1024))

            def emit_qk_weights(ranges):
                for dc in range(4):
                    for lo, hi in ranges:
                        nc.scalar.dma_start(
                            out=wqk_f32[:, dc, lo:hi],
                            in_=wqkv_d.ap()[dc * 128 : (dc + 1) * 128, lo:hi],
                        )

            def emit_qk_weight_conv(ranges, eng):
                for dpair in range(2):
                    for t in range(2):
                        for lo, hi in ranges:
                            eng(
                                out=w_qk_dr[:, dpair, t, lo:hi],
                                in_=wqk_f32[:, 2 * dpair + t, lo:hi],
                            )

            def emit_late_weights():
                # V-columns and output-projection weights: needed only after
                # the q/k projections, so they load in their shadow
                for dc in range(4):
                    nc.gpsimd.dma_start(
                        out=w_v[:, dc, :],
                        in_=wqkv_d.ap()[dc * 128 : (dc + 1) * 128, 1024:1536],
                    )
                nc.gpsimd.dma_start(
                    out=w_p[:],
                    in_=wproj_d.ap().rearrange("(c p) o -> p c o", p=128),
                )

            def emit_load_x(b):
                # b0 loads per dc-column-chunk so each transpose group can
                # start as soon as its quarter arrives; later batches have a
                # full slot of slack, so one DMA saves Q7 descriptor time
                xb = xbpool.tile([128, 4, D], BF16, tag="xb", name="xb")
                xsrc = x_d.ap()[b].rearrange("(st p) d -> p st d", p=128)
                if b == 0:
                    for dc in range(4):
                        nc.gpsimd.dma_start(
                            out=xb[:, :, dc * 128 : (dc + 1) * 128],
                            in_=xsrc[:, :, dc * 128 : (dc + 1) * 128],
                        )
                else:
                    nc.gpsimd.dma_start(out=xb[:], in_=xsrc)
                v_sb = vpool.tile([128, 4, H, 66], BF16, tag="v", name="vsb")
                nc.gpsimd.memset(v_sb[:, :, :, 64:65], 1.0)
                state[b] = {"xb": xb, "xT": [], "xdr": {}, "qkT": {}, "v": v_sb,
                            "ynT": []}

            def emit_xt_group(b, dc):
                st_ = state[b]
                tp = ps_mm.tile([128, 512], F32, tag="mm", name="tpx")
                tb = tp[:].bitcast(BF16)  # [128, 1024]
                for st in range(4):
                    nc.tensor.transpose(
                        tb[:, st * 128 : (st + 1) * 128],
                        st_["xb"][:, st, dc * 128 : (dc + 1) * 128],
                        ident[:],
                    )
                dpair = dc // 2
                if dc % 2 == 0:
                    st_["xdr"][dpair] = xdpool.tile(
                        [128, 2, 512], FP8, tag=f"xdr{dpair}", name=f"xdr{dpair}"
                    )
                nc.vector.tensor_copy(
                    out=st_["xdr"][dpair][:, dc % 2, :], in_=tb[:, 0:512]
                )
                xt = xtpool.tile([128, 512], BF16, tag=f"xt{dc}", name=f"xt{dc}")
                nc.vector.tensor_copy(out=xt[:], in_=tb[:, 0:512])
                st_["xT"].append(xt)

            def emit_qkt_group(b, mt):
                st_ = state[b]
                mp = ps_mm.tile([128, 512], F32, tag="mm", name="mp")
                for dpair in range(2):
                    nc.tensor.matmul(
                        mp[:],
                        w_qk_dr[:, dpair, :, mt * 128 : (mt + 1) * 128],
                        st_["xdr"][dpair][:],
                        start=(dpair == 0),
                        stop=(dpair == 1),
                        perf_mode=DR,
                    )
                qk = qkpool.tile([128, 512], BF16, tag=f"qk{mt}", name=f"qk{mt}")
                nc.vector.tensor_copy(out=qk[:], in_=mp[:])
                st_["qkT"][mt] = qk

            def emit_v_group(b, stt):
                st_ = state[b]
                vp_ps = ps_mm.tile([128, 512], F32, tag="mm", name="vp")
                for dc in range(4):
                    nc.tensor.matmul(
                        vp_ps[:],
                        st_["xT"][dc][:, stt * 128 : (stt + 1) * 128],
                        w_v[:, dc, :],
                        start=(dc == 0),
                        stop=(dc == 3),
                    )
                nc.scalar.copy(
                    out=st_["v"][:, stt, :, 0:64],
                    in_=vp_ps[:].rearrange("p (h e) -> p h e", h=H),
                )

            def proj_work(b):
                w = [partial(emit_load_x, b)]
                w += [partial(emit_xt_group, b, dc) for dc in range(4)]
                w += [partial(emit_qkt_group, b, mt) for mt in (0, 4, 1, 5, 2, 6, 3, 7)]
                w += [partial(emit_v_group, b, stt) for stt in range(4)]
                return w

            # ---------- attention ----------
            def emit_scores(b, hg):
                qkT = state[b]["qkT"]
                h0, h1 = 2 * hg, 2 * hg + 1
                se = sepool.tile([128, 4, 2, 512], BF16, tag="se", name="se")
                for kt in range(4):
                    off = 128 * kt
                    stp = ps_att.tile([128, 1024], F32, tag="att", name="stp")
                    for hi, h in enumerate((h0, h1)):
                        mtq, poq = h // 2, 64 * (h % 2)
                        mtk, pok = 4 + h // 2, 64 * (h % 2)
                        nc.tensor.matmul(
                            stp[:, hi * 512 + off : hi * 512 + 512],
                            qkT[mtk][pok : pok + 64, kt * 128 : (kt + 1) * 128],
                            qkT[mtq][poq : poq + 64, off:512],
                            start=True,
                            stop=True,
                        )
                    nc.scalar.activation(
                        out=se[:, kt, :, off:],
                        in_=stp[:].rearrange("p (hh q) -> p hh q", hh=2)[:, :, off:],
                        func=mybir.ActivationFunctionType.Exp,
                        scale=1.0 / np.sqrt(HD),
                    )
                    # mask the diagonal block (strict lower triangle -> 0)
                    nc.vector.tensor_mul(
                        out=se[:, kt, :, off : off + 128],
                        in0=se[:, kt, :, off : off + 128],
                        in1=tri2[:],
                    )
                return se

            def emit_y(b, hg, se, y_sb):
                st_ = state[b]
                h0, h1 = 2 * hg, 2 * hg + 1
                yp = ps_att.tile([128, 1024], F32, tag="att", name="yp")
                for hi, h in enumerate((h0, h1)):
                    for qt in range(4):
                        base = hi * 512 + qt * 65
                        for kt in range(qt + 1):
                            nc.tensor.matmul(
                                yp[:, base : base + 65],
                                se[:, kt, hi, qt * 128 : (qt + 1) * 128],
                                st_["v"][:, kt, h, 0:65],
                                start=(kt == 0),
                                stop=(kt == qt),
                            )
                rs = rpool.tile([128, 2, 4], F32, tag="rs", name="rs")
                nc.vector.reciprocal_approx_fast(
                    out=rs[:],
                    in_=yp[:].rearrange("p (hh q) -> p hh q", hh=2)[:, :, 64:260:65],
                )
                # normalize the whole head-pair with one broadcast tensor_tensor:
                # out[p, hh, qt, c] = yp[p, hh*512 + qt*65 + c] * rs[p, hh, qt]
                yp_v = bass.AP(
                    tensor=yp[:].tensor,
                    offset=yp[:].offset,
                    ap=[yp[:].ap[0], [512, 2], [65, 4], [1, 64]],
                )
                rs_v = bass.AP(
                    tensor=rs[:].tensor,
                    offset=rs[:].offset,
                    ap=[rs[:].ap[0], rs[:].ap[1], rs[:].ap[2], [0, 64]],
                )
                y_v = y_sb[:, :, hg * 128 : (hg + 1) * 128].rearrange(
                    "p q (hh c) -> p hh q c", hh=2
                )
                nc.vector.tensor_mul(out=y_v, in0=yp_v, in1=rs_v)

            def emit_y_tail(b, hg, y_sb):
                st_ = state[b]
                dc = hg
                tp = ps_mm.tile([128, 512], F32, tag="mm", name="tpy")
                tb = tp[:].bitcast(BF16)
                for qt in range(4):
                    nc.tensor.transpose(
                        tb[:, qt * 128 : (qt + 1) * 128],
                        y_sb[:, qt, dc * 128 : (dc + 1) * 128],
                        ident[:],
                    )
                yt = ytpool.tile([128, 512], BF16, tag=f"yt{dc}", name=f"yt{dc}")
                nc.scalar.copy(out=yt[:], in_=tb[:, 0:512])
                st_["ynT"].append(yt)

            def emit_proj_group(b, qt):
                ynT = state[b]["ynT"]
                op_ps = ps_mm.tile([128, 512], F32, tag="mm", name="op")
                for dc in range(4):
                    nc.tensor.matmul(
                        op_ps[:],
                        ynT[dc][:, qt * 128 : (qt + 1) * 128],
                        w_p[:, dc, :],
                        start=(dc == 0),
                        stop=(dc == 3),
                    )
                ob = opool.tile([128, 512], F32, tag="ob", name="ob")
                nc.vector.tensor_copy(out=ob[:], in_=op_ps[:])
                nc.sync.dma_start(
                    out=out_d.ap()[b, qt * 128 : (qt + 1) * 128, :], in_=ob[:]
                )

            # ---------- main schedule ----------
            w0 = proj_work(0)
            w0[0]()  # load x0 (SWDGE; weight DMAs go on the HWDGE queue)
            emit_qk_weights(QK_A)
            emit_qk_weight_conv(QK_A, nc.vector.tensor_copy)
            for f in w0[1:5]:
                f()  # xT groups
            w0[5]()  # qkT mt0
            w0[6]()  # qkT mt4
            emit_qk_weights(QK_B)
            emit_qk_weight_conv(QK_B, nc.vector.tensor_copy)
            emit_late_weights()
            se_prev = emit_scores(0, 0)
            for f in w0[7:]:
                f()
            pending_proj = []
            for b in range(B):
                if b + 1 < B:
                    w = proj_work(b + 1)
                    queue = w[0:1] + pending_proj + w[1:]
                    plan = (1, 4, 6, 6) if b == 0 else (3, 4, 6, 6)
                else:
                    queue = list(pending_proj)
                    plan = (1, 1, 1, 1)
                y_sb = ypool.tile([128, 4, 512], BF16, tag="y", name="ysb")
                for hg in range(4):
                    se_next = emit_scores(b, hg + 1) if hg + 1 < 4 else None
                    for _ in range(plan[hg]):
                        if queue:
                            queue.pop(0)()
                    if hg > 0:
                        emit_y_tail(b, hg - 1, y_sb)
                    emit_y(b, hg, se_prev, y_sb)
                    se_prev = se_next
                while queue:
                    queue.pop(0)()
                emit_y_tail(b, 3, y_sb)
                # first scores of the next batch fill the normalize/transpose tail
                se_prev = emit_scores(b + 1, 0) if b + 1 < B else None
                # this batch's projection is deferred into the next attention
                pending_proj = [partial(emit_proj_group, b, qt) for qt in range(4)]
            for f in pending_proj:
                f()

    nc.compile()
    return nc


def kernel(x, qkv_weight, proj_weight):
    if "nc" not in _cache:
        _cache["nc"] = build_nc()
    nc = _cache["nc"]
    in_maps = [
        {
            "x": np.ascontiguousarray(x[m], dtype=np.float32),
            "wqkv": np.ascontiguousarray(qkv_weight[m], dtype=np.float32),
            "wproj": np.ascontiguousarray(proj_weight[m], dtype=np.float32),
        }
        for m in range(M)
    ]
    res = bass_utils.run_bass_kernel_spmd(nc, in_maps, core_ids=list(range(N_CORES)))
    return np.stack([res.results[m]["out"] for m in range(M)]).astype(np.float32)


# revision 30
# speedup vs baseline: 1.2982x; 1.0097x over previous
"""Bass/Tile TRN2 kernel for per-model-batched causal self-attention.

Problem: x[M,B,S,D], qkv_w[M,D,3D], proj_w[M,D,D] -> out[M,B,S,D]
M=8 models sharded across 8 NeuronCores (embarrassingly parallel).

Per-core dataflow (model m, per batch b):
  xb      = cast-DMA x -> bf16                        [S,D] bf16
  xT      = PE-transpose(xb) (bf16, 1 cyc/row)        [D,S] bf16
  xdr     = fp8(xT) interleaved [128, 2, 512] pairs for DoubleRow
  qkT     = fp8 DoubleRow matmul w_qk_dr.T @ xdr      [1024,S] -> bf16
            (2 accumulation steps of 256-contraction, 0.5 cyc/row)
  V       = xT.T @ w_v (bf16)                         [S,512] bf16, +ones col
  st[k,q] = K @ Q^T (bf16, causal-trimmed, head pairs row-packed)
  p       = exp(st/8) (ScalarE), diag blocks masked by tri01 mul (DVE)
  y_aug   = p.T @ V_aug (bf16) -> y[q,d] + softmax sums in col 64
  y       = y_aug * (1/sums)  one broadcast tensor_tensor per head pair
  ynT     = PE-transpose(y) (bf16)                    [D,S]
  out     = ynT.T @ w_p (bf16)

Scheduling: the next batch's load/transpose/projection groups and the
previous batch's output projections are interleaved into the attention
loop as a work queue, paced (pop plan per head-pair) so each group lands
after its inputs exist and the in-order PE never blocks on a producer.
Each head-pair's yT transposes are deferred one slot so the PE never
waits on the recip/normalize chain. q/k weights load as f32 on the HWDGE
queue (parallel with the SWDGE x casts), split so the mt0/mt4 columns
convert to fp8 first and unblock the first score tile. The fp8 xdr pairs
are copied straight out of the transpose PSUM on DVE (before the bf16 xT
copy) so the DoubleRow q/k matmuls never wait on a two-hop DVE->Pool
chain.
"""

import sys

if "/opt/trn_rl_repo" not in sys.path:
    sys.path.insert(0, "/opt/trn_rl_repo")

from contextlib import nullcontext
from functools import partial

import numpy as np

import concourse.bass as bass
import concourse.mybir as mybir
import concourse.tile as tile
from concourse import bacc, bass_utils
from concourse.masks import make_identity, make_upper_triangular

M, B, S, D, H = 8, 4, 512, 512, 8
HD = D // H  # 64
F32 = mybir.dt.float32
BF16 = mybir.dt.bfloat16
FP8 = mybir.dt.float8e4
DR = mybir.MatmulPerfMode.DoubleRow

N_CORES = 8

_cache = {}


def build_nc(reps=1):
    nc = bacc.Bacc("TRN2", target_bir_lowering=False, debug=False)

    x_d = nc.dram_tensor("x", [B, S, D], F32, kind="ExternalInput")
    wqkv_d = nc.dram_tensor("wqkv", [D, 3 * D], F32, kind="ExternalInput")
    wproj_d = nc.dram_tensor("wproj", [D, D], F32, kind="ExternalInput")
    out_d = nc.dram_tensor("out", [B, S, D], F32, kind="ExternalOutput")

    with tile.TileContext(nc) as tc:
        with (
            tc.tile_pool(name="singles", bufs=1) as singles,
            tc.tile_pool(name="xbp", bufs=2) as xbpool,
            tc.tile_pool(name="xtp", bufs=2) as xtpool,
            tc.tile_pool(name="xdp", bufs=2) as xdpool,
            tc.tile_pool(name="qk", bufs=2) as qkpool,
            tc.tile_pool(name="vp", bufs=2) as vpool,
            tc.tile_pool(name="se", bufs=3) as sepool,
            tc.tile_pool(name="yp", bufs=2) as ypool,
            tc.tile_pool(name="ytp", bufs=3) as ytpool,
            tc.tile_pool(name="op", bufs=3) as opool,
            tc.tile_pool(name="rp", bufs=4) as rpool,
            tc.tile_pool(name="ps_mm", bufs=2, space=bass.MemorySpace.PSUM) as ps_mm,
            tc.tile_pool(name="ps_att", bufs=3, space=bass.MemorySpace.PSUM) as ps_att,
        ):
          with tc.For_i(0, reps, 1) if reps > 1 else nullcontext():
            # ---- constants ----
            identf = singles.tile([128, 128], F32)
            make_identity(nc, identf[:])
            ident = singles.tile([128, 128], BF16)
            nc.vector.tensor_copy(out=ident[:], in_=identf[:])
            tri2 = singles.tile([128, 2, 128], BF16)  # keep-mask (k<=q), x2 heads
            make_upper_triangular(nc, tri2[:, 0, :], val=1.0, diag=True)
            nc.gpsimd.tensor_copy(out=tri2[:, 1, :], in_=tri2[:, 0, :])

            wqk_f32 = singles.tile([128, 4, 1024], F32)
            w_qk_dr = singles.tile([128, 2, 2, 1024], FP8)
            w_v = singles.tile([128, 4, 512], BF16)
            w_p = singles.tile([128, 4, 512], BF16)

            state = {}

            # qk-weight column halves: A = mt chunks 0 and 4 (unblocks the
            # first score tile), B = the rest. f32 via HWDGE, parallel with
            # the SWDGE x casts; fp8 conversion on DVE.
            QK_A = ((0, 128), (512, 640))
            QK_B = ((128, 512), (640, 1024))

            def emit_qk_weights(ranges):
                for dc in range(4):
                    for lo, hi in ranges:
                        nc.scalar.dma_start(
                            out=wqk_f32[:, dc, lo:hi],
                            in_=wqkv_d.ap()[dc * 128 : (dc + 1) * 128, lo:hi],
                        )

            def emit_qk_weight_conv(ranges, eng):
                for dpair in range(2):
                    for t in range(2):
                        for lo, hi in ranges:
                            eng(
                                out=w_qk_dr[:, dpair, t, lo:hi],
                                in_=wqk_f32[:, 2 * dpair + t, lo:hi],
                            )

            def emit_late_weights():
                # V-columns and output-projection weights: needed only after
                # the q/k projections, so they load in their shadow
                for dc in range(4):
                    nc.gpsimd.dma_start(
                        out=w_v[:, dc, :],
                        in_=wqkv_d.ap()[dc * 128 : (dc + 1) * 128, 1024:1536],
                    )
                nc.gpsimd.dma_start(
                    out=w_p[:],
                    in_=wproj_d.ap().rearrange("(c p) o -> p c o", p=128),
                )

            def emit_load_x(b):
                # b0 loads per dc-column-chunk so each transpose group can
                # start as soon as its quarter arrives; later batches have a
                # full slot of slack, so one DMA saves Q7 descriptor time
                xb = xbpool.tile([128, 4, D], BF16, tag="xb", name="xb")
                xsrc = x_d.ap()[b].rearrange("(st p) d -> p st d", p=128)
                if b == 0:
                    for dc in range(4):
                        nc.gpsimd.dma_start(
                            out=xb[:, :, dc * 128 : (dc + 1) * 128],
                            in_=xsrc[:, :, dc * 128 : (dc + 1) * 128],
                        )
                else:
                    nc.gpsimd.dma_start(out=xb[:], in_=xsrc)
                v_sb = vpool.tile([128, 4, H, 66], BF16, tag="v", name="vsb")
                nc.gpsimd.memset(v_sb[:, :, :, 64:65], 1.0)
                state[b] = {"xb": xb, "xT": [], "xdr": {}, "qkT": {}, "v": v_sb,
                            "ynT": []}

            def emit_xt_group(b, dc):
                st_ = state[b]
                tp = ps_mm.tile([128, 512], F32, tag="mm", name="tpx")
                tb = tp[:].bitcast(BF16)  # [128, 1024]
                for st in range(4):
                    nc.tensor.transpose(
                        tb[:, st * 128 : (st + 1) * 128],
                        st_["xb"][:, st, dc * 128 : (dc + 1) * 128],
                        ident[:],
                    )
                dpair = dc // 2
                if dc % 2 == 0:
                    st_["xdr"][dpair] = xdpool.tile(
                        [128, 2, 512], FP8, tag=f"xdr{dpair}", name=f"xdr{dpair}"
                    )
                nc.vector.tensor_copy(
                    out=st_["xdr"][dpair][:, dc % 2, :], in_=tb[:, 0:512]
                )
                xt = xtpool.tile([128, 512], BF16, tag=f"xt{dc}", name=f"xt{dc}")
                nc.vector.tensor_copy(out=xt[:], in_=tb[:, 0:512])
                st_["xT"].append(xt)

            def emit_qkt_group(b, mt):
                st_ = state[b]
                mp = ps_mm.tile([128, 512], F32, tag="mm", name="mp")
                for dpair in range(2):
                    nc.tensor.matmul(
                        mp[:],
                        w_qk_dr[:, dpair, :, mt * 128 : (mt + 1) * 128],
                        st_["xdr"][dpair][:],
                        start=(dpair == 0),
                        stop=(dpair == 1),
                        perf_mode=DR,
                    )
                qk = qkpool.tile([128, 512], BF16, tag=f"qk{mt}", name=f"qk{mt}")
                nc.vector.tensor_copy(out=qk[:], in_=mp[:])
                st_["qkT"][mt] = qk

            def emit_v_group(b, stt):
                st_ = state[b]
                vp_ps = ps_mm.tile([128, 512], F32, tag="mm", name="vp")
                for dc in range(4):
                    nc.tensor.matmul(
                        vp_ps[:],
                        st_["xT"][dc][:, stt * 128 : (stt + 1) * 128],
                        w_v[:, dc, :],
                        start=(dc == 0),
                        stop=(dc == 3),
                    )
                nc.scalar.copy(
                    out=st_["v"][:, stt, :, 0:64],
                    in_=vp_ps[:].rearrange("p (h e) -> p h e", h=H),
                )

            def proj_work(b):
                w = [partial(emit_load_x, b)]
                w += [partial(emit_xt_group, b, dc) for dc in range(4)]
                for i, (m0, m1) in enumerate(((0, 4), (1, 5), (2, 6), (3, 7))):
                    w.append(partial(emit_qkt_group, b, m0))
                    w.append(partial(emit_qkt_group, b, m1))
                    w.append(partial(emit_v_group, b, i))
                return w

            # ---------- attention ----------
            def emit_scores(b, hg):
                qkT = state[b]["qkT"]
                h0, h1 = 2 * hg, 2 * hg + 1
                se = sepool.tile([128, 4, 2, 512], BF16, tag="se", name="se")
                for kt in range(4):
                    off = 128 * kt
                    stp = ps_att.tile([128, 1024], F32, tag="att", name="stp")
                    for hi, h in enumerate((h0, h1)):
                        mtq, poq = h // 2, 64 * (h % 2)
                        mtk, pok = 4 + h // 2, 64 * (h % 2)
                        nc.tensor.matmul(
                            stp[:, hi * 512 + off : hi * 512 + 512],
                            qkT[mtk][pok : pok + 64, kt * 128 : (kt + 1) * 128],
                            qkT[mtq][poq : poq + 64, off:512],
                            start=True,
                            stop=True,
                        )
                    nc.scalar.activation(
                        out=se[:, kt, :, off:],
                        in_=stp[:].rearrange("p (hh q) -> p hh q", hh=2)[:, :, off:],
                        func=mybir.ActivationFunctionType.Exp,
                        scale=1.0 / np.sqrt(HD),
                    )
                    # mask the diagonal block (strict lower triangle -> 0)
                    nc.vector.tensor_mul(
                        out=se[:, kt, :, off : off + 128],
                        in0=se[:, kt, :, off : off + 128],
                        in1=tri2[:],
                    )
                return se

            def emit_y(b, hg, se, y_sb):
                st_ = state[b]
                h0, h1 = 2 * hg, 2 * hg + 1
                yp = ps_att.tile([128, 1024], F32, tag="att", name="yp")
                for hi, h in enumerate((h0, h1)):
                    for qt in range(4):
                        base = hi * 512 + qt * 65
                        for kt in range(qt + 1):
                            nc.tensor.matmul(
                                yp[:, base : base + 65],
                                se[:, kt, hi, qt * 128 : (qt + 1) * 128],
                                st_["v"][:, kt, h, 0:65],
                                start=(kt == 0),
                                stop=(kt == qt),
                            )
                rs = rpool.tile([128, 2, 4], F32, tag="rs", name="rs")
                nc.vector.reciprocal_approx_fast(
                    out=rs[:],
                    in_=yp[:].rearrange("p (hh q) -> p hh q", hh=2)[:, :, 64:260:65],
                )
                # normalize the whole head-pair with one broadcast tensor_tensor:
                # out[p, hh, qt, c] = yp[p, hh*512 + qt*65 + c] * rs[p, hh, qt]
                yp_v = bass.AP(
                    tensor=yp[:].tensor,
                    offset=yp[:].offset,
                    ap=[yp[:].ap[0], [512, 2], [65, 4], [1, 64]],
                )
                rs_v = bass.AP(
                    tensor=rs[:].tensor,
                    offset=rs[:].offset,
                    ap=[rs[:].ap[0], rs[:].ap[1], rs[:].ap[2], [0, 64]],
                )
                y_v = y_sb[:, :, hg * 128 : (hg + 1) * 128].rearrange(
                    "p q (hh c) -> p hh q c", hh=2
                )
                nc.vector.tensor_mul(out=y_v, in0=yp_v, in1=rs_v)

            def emit_y_tail(b, hg, y_sb):
                st_ = state[b]
                dc = hg
                tp = ps_mm.tile([128, 512], F32, tag="mm", name="tpy")
                tb = tp[:].bitcast(BF16)
                for qt in range(4):
                    nc.tensor.transpose(
                        tb[:, qt * 128 : (qt + 1) * 128],
                        y_sb[:, qt, dc * 128 : (dc + 1) * 128],
                        ident[:],
                    )
                yt = ytpool.tile([128, 512], BF16, tag=f"yt{dc}", name=f"yt{dc}")
                nc.scalar.copy(out=yt[:], in_=tb[:, 0:512])
                st_["ynT"].append(yt)

            def emit_proj_group(b, qt):
                ynT = state[b]["ynT"]
                op_ps = ps_mm.tile([128, 512], F32, tag="mm", name="op")
                for dc in range(4):
                    nc.tensor.matmul(
                        op_ps[:],
                        ynT[dc][:, qt * 128 : (qt + 1) * 128],
                        w_p[:, dc, :],
                        start=(dc == 0),
                        stop=(dc == 3),
                    )
                ob = opool.tile([128, 512], F32, tag="ob", name="ob")
                nc.vector.tensor_copy(out=ob[:], in_=op_ps[:])
                nc.sync.dma_start(
                    out=out_d.ap()[b, qt * 128 : (qt + 1) * 128, :], in_=ob[:]
                )

            # ---------- main schedule ----------
            w0 = proj_work(0)
            w0[0]()  # load x0 (SWDGE; weight DMAs go on the HWDGE queue)
            emit_qk_weights(QK_A)
            emit_qk_weight_conv(QK_A, nc.vector.tensor_copy)
            for f in w0[1:5]:
                f()  # xT groups
            w0[5]()  # qkT mt0
            w0[6]()  # qkT mt4
            emit_qk_weights(QK_B)
            emit_qk_weight_conv(QK_B, nc.vector.tensor_copy)
            emit_late_weights()
            se_prev = emit_scores(0, 0)
            for f in w0[7:]:
                f()
            pending_proj = []
            for b in range(B):
                if b + 1 < B:
                    w = proj_work(b + 1)
                    queue = w[0:1] + pending_proj + w[1:]
                    plan = (1, 4, 6, 6) if b == 0 else (3, 4, 6, 6)
                else:
                    queue = list(pending_proj)
                    plan = (1, 1, 1, 1)
                y_sb = ypool.tile([128, 4, 512], BF16, tag="y", name="ysb")
                for hg in range(4):
                    se_next = emit_scores(b, hg + 1) if hg + 1 < 4 else None
                    for _ in range(plan[hg]):
                        if queue:
                            queue.pop(0)()
                    if hg > 0:
                        emit_y_tail(b, hg - 1, y_sb)
                    emit_y(b, hg, se_prev, y_sb)
                    se_prev = se_next
                while queue:
                    queue.pop(0)()
                emit_y_tail(b, 3, y_sb)
                # first scores of the next batch fill the normalize/transpose tail
                se_prev = emit_scores(b + 1, 0) if b + 1 < B else None
                # this batch's projection is deferred into the next attention
                pending_proj = [partial(emit_proj_group, b, qt) for qt in range(4)]
            for f in pending_proj:
                f()

    nc.compile()
    return nc


def kernel(x, qkv_weight, proj_weight):
    if "nc" not in _cache:
        _cache["nc"] = build_nc()
    nc = _cache["nc"]
    in_maps = [
        {
            "x": np.ascontiguousarray(x[m], dtype=np.float32),
            "wqkv": np.ascontiguousarray(qkv_weight[m], dtype=np.float32),
            "wproj": np.ascontiguousarray(proj_weight[m], dtype=np.float32),
        }
        for m in range(M)
    ]
    res = bass_utils.run_bass_kernel_spmd(nc, in_maps, core_ids=list(range(N_CORES)))
    return np.stack([res.results[m]["out"] for m in range(M)]).astype(np.float32)
